# revision 48
# baseline (speedup 1.0000x reference)
"""Trainium2 Bass kernel for the 2-layer GraphSAGE encoder (mean aggregation).

Computation (see reference):
  h   = relu(mean_agg(relu(x)[src] by dst) @ W_l1 + b_l1 + x @ W_r1)
  out =      mean_agg(h[src]       by dst) @ W_l2 + b_l2 + h @ W_r2

Distribution (src-sharded): nodes are sliced 1/8 per core; each core keeps
its slice's features resident in SBUF and handles exactly the edges whose
src falls in its slice.  Messages are fetched with SBUF-source dma_gather
(transpose mode -> feature-major chunks), transformed by W_l on the tensor
engine (linearity lets W_l and the 1/deg mean scale commute with the
segment sum), aggregated per 128-node dst window via one-hot matmuls into
PSUM, and written as bf16 partial sums for all N nodes.  Partial sums are
reduced across cores by PIPELINED sub-ReduceScatters: windows are processed
in an order that completes one cross-core block at a time, and each block's
collective fires as soon as its rows land in DRAM, overlapping the
collective with the remaining aggregation.  The combine (mean@W_l root
z@W_r + bias + activation) then runs purely per-slice, and layer 2 reuses
the resulting h slice as its gather table - no AllGather needed anywhere.

Host prep computes global in-degrees (the mean scale is folded into the
per-window flush as an activation scale) and per-core FLAT-PACKED edge
streams (dense 128-slot chunks, cores realigned only every WGRP windows,
~3% pad vs ~33% for per-window padding): int16 gather indices (wrap-16
layout) plus per-(chunk,window) uint8 dst-lane columns that drive is_equal
one-hot builds on the DVE, so per-core window straddling stays data-driven
while all cores share one instruction stream.
"""
import os
import sys

sys.path.insert(0, "/opt/trn_rl_repo")

import numpy as np
import ml_dtypes

import concourse.bacc as bacc
import concourse.tile as tile
from concourse import bass, mybir
from concourse.bass_utils import run_bass_kernel_spmd
from concourse.masks import make_identity

F32 = mybir.dt.float32
BF16 = mybir.dt.bfloat16
F8 = mybir.dt.float8e4
U8 = mybir.dt.uint8
I16 = mybir.dt.int16
BF = ml_dtypes.bfloat16
F8NP = ml_dtypes.float8_e4m3

P = 128
D = 128
NCORES = 8
PAD_LANE = 255.0   # doff value for pad slots (no iota lane matches)
G = 16             # chunks per dma_gather instruction
SUB = 4            # chunks per transform-psum batch
WB = 7             # windows per staging DMA batch

LAST_EXEC_NS = None
LAST_RESULTS = None
LAST_NC = None
LAST_IN_MAPS = None
LAST_CFG = None
LAST_PREP = None


class Cfg:
    def __init__(self, n_nodes, n_edges):
        assert n_nodes % NCORES == 0
        self.N = n_nodes
        self.E = n_edges
        self.NSH = n_nodes // NCORES            # real nodes per slice
        self.NWS = -(-self.NSH // P)            # windows per slice
        self.NSHP = self.NWS * P                # padded nodes per slice
        self.NW = NCORES * self.NWS             # global windows
        self.NPAD = NCORES * self.NSHP          # padded global nodes
        assert self.NSHP - 1 <= 32767


WGRP = 16          # windows per realignment group (flat-packed chunks inside)


def _host_prep(cfg, x, edge_index):
    """Flat-packed edge streams with periodic realignment.

    Edges are window-major sorted per core, then packed densely into
    128-slot chunks.  Cores realign only at WGRP-window group boundaries
    (pad-to-common-chunk-count there, ~3% waste), so a chunk may straddle
    window boundaries at per-core-varying positions.  The aggregation
    schedule is a flat list of ops (chunk, window, start, stop): each op
    builds a one-hot sel from its own doff column (255 = no-match for
    slots outside that window on that core) and matmuls into the window's
    PSUM accumulator, so per-core routing stays data-driven while the
    instruction stream is shared.
    """
    src = np.asarray(edge_index[0], dtype=np.int64)
    dst = np.asarray(edge_index[1], dtype=np.int64)
    E = src.shape[0]

    core = src // cfg.NSH
    idx16 = (src % cfg.NSH).astype(np.int16)
    pd = (dst // cfg.NSH) * cfg.NSHP + (dst % cfg.NSH)
    worig = pd // P
    lane = (pd % P).astype(np.int64)

    NW, NWS = cfg.NW, cfg.NWS
    # pi-order: windows processed grouped for pipelined sub-ReduceScatter.
    # RS group j covers local windows gs_lo[j]..gs_hi[j] of EVERY dst core;
    # processing order (j, dst_core, l) makes each group's PART rows complete
    # (and contiguous) early, so its collective can fire while later groups
    # still aggregate.  RSOUT comes out in plain l-ascending order per core.
    nsub = max(1, min(int(os.environ.get("GNN_NSUB", "7")), NWS))
    q, rem = divmod(NWS, nsub)
    gsizes = [q + (1 if j < rem else 0) for j in range(nsub)]
    pi_list = []                  # pi-rank -> global window id
    for j in range(nsub):
        lo = sum(gsizes[:j])
        for d in range(NCORES):
            for l in range(lo, lo + gsizes[j]):
                pi_list.append(d * NWS + l)
    rank_of_w = np.empty(NW, np.int64)
    rank_of_w[np.asarray(pi_list)] = np.arange(NW)
    grp_end = np.cumsum([NCORES * g for g in gsizes]) - 1   # pi-rank of group end

    w = rank_of_w[worig]          # everything downstream uses pi-ranks
    ngrp = -(-NW // WGRP)
    grp = w // WGRP

    # per-(core, group) counts -> chunks per group = max over cores
    cg = core * ngrp + grp
    counts = np.bincount(cg, minlength=NCORES * ngrp).reshape(NCORES, ngrp)
    Kg = np.maximum(-(-counts.max(axis=0) // P), 1)     # [ngrp] chunks
    gbase = np.zeros(ngrp + 1, np.int64)
    np.cumsum(Kg, out=gbase[1:])
    nch = int(gbase[-1])
    slots = nch * P

    # slot of each edge: window-major order within its (core, group)
    key = (core * ngrp + grp) * NW + w
    order = np.argsort(key, kind="stable")
    ks_cg = cg[order]
    gs = np.searchsorted(ks_cg, np.arange(NCORES * ngrp), side="left")
    rank = np.arange(E) - gs[ks_cg]
    slot = gbase[ks_cg % ngrp] * P + rank
    ecore = ks_cg // ngrp

    idx_st = np.zeros((NCORES, slots), np.int16)
    win_st = np.full((NCORES, slots), -1, np.int64)
    lan_st = np.full((NCORES, slots), int(PAD_LANE), np.uint8)
    idx_st[ecore, slot] = idx16[order]
    win_st[ecore, slot] = w[order]
    lan_st[ecore, slot] = lane[order].astype(np.uint8)

    # ops: per chunk, the union (over cores) of windows present
    wc = win_st.reshape(NCORES, nch, P)
    lc = lan_st.reshape(NCORES, nch, P)
    chunk_ws = []                 # per chunk: sorted windows present in any core
    for k in range(nch):
        ws = np.unique(wc[:, k, :])
        chunk_ws.append([int(v) for v in ws if v >= 0])
    # every window must flush exactly once; attach empty windows (possible
    # only in tiny graphs) to the first chunk of the next present window
    present = {wv for ws in chunk_ws for wv in ws}
    for wv in range(NW):
        if wv not in present:
            k = next((k for k in range(nch)
                      if any(x > wv for x in chunk_ws[k])), nch - 1)
            chunk_ws[k].append(wv)
            chunk_ws[k].sort()
    ops = []                      # [chunk, window, start, stop]
    first_op = {}
    last_op = {}
    for k in range(nch):
        for wv in chunk_ws[k]:
            op = len(ops)
            ops.append([k, wv, False, False])
            if wv not in first_op:
                first_op[wv] = op
            last_op[wv] = op
    for wv, op in first_op.items():
        ops[op][2] = True
    for wv, op in last_op.items():
        ops[op][3] = True
    nops = len(ops)

    doffc = np.full((NCORES, P, nops), int(PAD_LANE), np.uint8)
    for o, (k, wv, _, _) in enumerate(ops):
        m = wc[:, k, :] == wv                            # [8, 128]
        col = np.where(m, lc[:, k, :], int(PAD_LANE))
        doffc[:, :, o] = col

    idxw = np.ascontiguousarray(
        idx_st.reshape(NCORES, slots // 16, 16).transpose(0, 2, 1)
    )                                                   # [8, 16, slots/16]

    indeg = np.bincount(dst, minlength=cfg.N).astype(np.float64)
    rv = (1.0 / np.maximum(indeg, 1.0)).astype(np.float32)
    rvp = np.ones(cfg.NPAD, np.float32)
    nodes = np.arange(cfg.N)
    rvp[(nodes // cfg.NSH) * cfg.NSHP + nodes % cfg.NSH] = rv
    rvq = np.ascontiguousarray(
        rvp.reshape(cfg.NW, P)[np.asarray(pi_list)].T
    ).astype(BF)                                        # [128, NW] pi-order

    xsl = np.zeros((NCORES, cfg.NSHP, D), BF)
    xs = np.asarray(x, np.float32).reshape(NCORES, cfg.NSH, D)
    xsl[:, : cfg.NSH] = xs

    return dict(ops=ops, nch=nch, idxw=idxw, doffc=doffc, rvq=rvq, xsl=xsl,
                grp_end=[int(v) for v in grp_end], gs=gsizes)


def _build_program(cfg, prep):
    ops, nch = prep["ops"], prep["nch"]
    grp_end, gs = prep["grp_end"], prep["gs"]
    skip = os.environ.get("GNN_SKIP", "")
    gg = int(os.environ.get("GNN_G", G))
    gsp = bool(int(os.environ.get("GNN_SP", "0")))
    nrep = int(os.environ.get("GNN_REPEAT", "1"))
    nq = int(os.environ.get("GNN_NQ", "1"))
    nc = bacc.Bacc(None, target_bir_lowering=False, debug=False,
                   num_swdge_queues=nq)
    slots = nch * P
    nops = len(ops)
    NWS, NW = cfg.NWS, cfg.NW
    ops_by_chunk = {}
    for oi, (k, wv, st, sp) in enumerate(ops):
        ops_by_chunk.setdefault(k, []).append((oi, wv, st, sp))

    xsl_t = nc.declare_dram_parameter("xmy", [cfg.NSHP, D], BF16, isOutput=False)
    idxw_t = nc.declare_dram_parameter("idxw", [16, slots // 16], I16, isOutput=False)
    doff_t = nc.declare_dram_parameter("doffc", [P, nops], U8, isOutput=False)
    rvq_t = nc.declare_dram_parameter("rvq", [P, NW], BF16, isOutput=False)
    iota_t = nc.declare_dram_parameter("iota", [P, P], BF16, isOutput=False)
    wl1_t = nc.declare_dram_parameter("W_l1", [D, D], BF16, isOutput=False)
    wr1_t = nc.declare_dram_parameter("W_r1", [D, D], BF16, isOutput=False)
    wl2_t = nc.declare_dram_parameter("W_l2", [D, D], BF16, isOutput=False)
    wr2_t = nc.declare_dram_parameter("W_r2", [D, D], BF16, isOutput=False)
    bt1_t = nc.declare_dram_parameter("bt1", [P, P], BF16, isOutput=False)
    bt2_t = nc.declare_dram_parameter("bt2", [P, P], BF16, isOutput=False)
    out_t = nc.declare_dram_parameter("out", [cfg.NSHP, D], BF16, isOutput=True)

    Id = mybir.ActivationFunctionType.Identity
    Relu = mybir.ActivationFunctionType.Relu

    with tile.TileContext(nc, trace_sim=bool(os.environ.get("GNN_TRACE_SIM"))) as tc:
        with (
            tc.tile_pool(name="const", bufs=1) as cp,
            tc.tile_pool(name="gather", bufs=3) as gp,
            tc.tile_pool(name="mk", bufs=3) as mp,
            tc.tile_pool(name="sel", bufs=8) as op_,
            tc.tile_pool(name="stage", bufs=2) as sp_,
            tc.tile_pool(name="rs", bufs=2) as rp,
            tc.tile_pool(name="tf", bufs=2, space="PSUM") as tfp,
            tc.tile_pool(name="win", bufs=3, space="PSUM") as wpp,
            tc.tile_pool(name="cpp", bufs=1, space="PSUM") as cpp,
            tc.tile_pool(name="tps", bufs=2, space="PSUM") as tpp,
            tc.tile_pool(name="dram", bufs=1, space="DRAM") as dp,
        ):
            ident = cp.tile([P, P], BF16)
            make_identity(nc, ident[:])
            ones_t = cp.tile([P, P], BF16)
            nc.vector.memset(ones_t[:], 1.0)
            iota_s = cp.tile([P, P], BF16)
            nc.sync.dma_start(iota_s[:], iota_t[:, :])
            wl1 = cp.tile([D, D], BF16)
            nc.sync.dma_start(wl1[:], wl1_t[:, :])
            wr1 = cp.tile([D, D], BF16)
            nc.sync.dma_start(wr1[:], wr1_t[:, :])
            wl2 = cp.tile([D, D], BF16)
            nc.sync.dma_start(wl2[:], wl2_t[:, :])
            wr2 = cp.tile([D, D], BF16)
            nc.sync.dma_start(wr2[:], wr2_t[:, :])
            bt1 = cp.tile([P, P], BF16)
            nc.sync.dma_start(bt1[:], bt1_t[:, :])
            bt2 = cp.tile([P, P], BF16)
            nc.sync.dma_start(bt2[:], bt2_t[:, :])
            rvq_b = cp.tile([P, NW], BF16)
            nc.sync.dma_start(rvq_b[:], rvq_t[:, :])
            rvq_s = cp.tile([P, NW], F32)
            nc.vector.tensor_copy(rvq_s[:], rvq_b[:])
            doff_b = cp.tile([P, nops], U8)
            nc.sync.dma_start(doff_b[:], doff_t[:, :])
            doff_s = cp.tile([P, nops], F32)
            nc.vector.tensor_copy(doff_s[:], doff_b[:])
            idx_s = cp.tile([P, slots // 16], I16)
            for r in range(8):
                nc.sync.dma_start(idx_s[16 * r : 16 * (r + 1), :], idxw_t[:, :])
            tbl1 = cp.tile([P, NWS * D], BF16)
            nc.sync.dma_start(
                tbl1[:].rearrange("t (r e) -> t r e", e=D),
                xsl_t[:, :].rearrange("(r t) e -> t r e", t=P),
            )
            tc.strict_bb_all_engine_barrier()

            # feature-major views of the slice (roots), gather table for L2
            xT = cp.tile([P, NWS * P], BF16)
            hT = cp.tile([P, NWS * P], BF16)
            tbl2 = cp.tile([P, NWS * D], BF16)
            for w in range(NWS):
                tp = tpp.tile([P, P], BF16, tag="tps", space="PSUM")
                nc.tensor.transpose(
                    out=tp[:], in_=tbl1[:, w * P : (w + 1) * P], identity=ident[:]
                )
                nc.scalar.activation(xT[:, w * P : (w + 1) * P], tp[:], Id)
            # messages are relu(x): relu the gather table in place (roots
            # already captured in xT)
            nc.scalar.activation(tbl1[:], tbl1[:], Relu)

            PART = dp.tile([cfg.NPAD, D], BF16, name="part")
            RSOUT = dp.tile([cfg.NSHP, D], BF16, name="rsout")

            for layer in [l for _ in range(nrep) for l in (1, 2)]:
                tbl = tbl1 if layer == 1 else tbl2
                zT = xT if layer == 1 else hT
                wl = wl1 if layer == 1 else wl2
                wr = wr1 if layer == 1 else wr2
                bt = bt1 if layer == 1 else bt2

                # combine one RS group's windows (l-range of own slice);
                # called interleaved with phase A once the group's sub-RS
                # has fired, so the PE/ACT combine work hides under the
                # gather-bound aggregation of later groups
                def emit_combine(j):
                    if "C" in skip:
                        return
                    glo = sum(gs[:j])
                    for q0 in range(glo, glo + gs[j], WB):
                        qc = min(WB, glo + gs[j] - q0)
                        rs = rp.tile([P, WB * P], BF16, tag="rs", name="rs")
                        nc.sync.dma_start(
                            rs[:, : qc * P].rearrange("l (w g) -> l w g",
                                                      g=D),
                            RSOUT[q0 * P : (q0 + qc) * P, :]
                            .rearrange("(w l) g -> l w g", l=P),
                        )
                        if layer == 2:
                            ost = sp_.tile([P, WB * P], BF16, tag="ost",
                                           name="ost")
                        else:
                            ost = None
                        for i in range(qc):
                            w = q0 + i
                            cps = cpp.tile([P, P], F32, tag="cps",
                                           name="cps", space="PSUM")
                            nc.tensor.matmul(
                                out=cps[:], lhsT=ident[:],
                                rhs=rs[:, i * P : (i + 1) * P],
                                start=True, stop=False,
                                skip_group_check=True,
                            )
                            nc.tensor.matmul(
                                out=cps[:], lhsT=ones_t[:], rhs=bt[:],
                                start=False, stop=False,
                                skip_group_check=True,
                            )
                            nc.tensor.matmul(
                                out=cps[:], lhsT=zT[:, w * P : (w + 1) * P],
                                rhs=wr[:],
                                start=False, stop=True,
                                skip_group_check=True,
                            )
                            if layer == 1:
                                nc.scalar.activation(
                                    tbl2[:, w * P : (w + 1) * P], cps[:],
                                    Relu
                                )
                                tp = tpp.tile([P, P], BF16, tag="tps",
                                              space="PSUM")
                                nc.tensor.transpose(
                                    out=tp[:],
                                    in_=tbl2[:, w * P : (w + 1) * P],
                                    identity=ident[:],
                                )
                                nc.scalar.activation(
                                    hT[:, w * P : (w + 1) * P], tp[:], Id
                                )
                            else:
                                nc.scalar.activation(
                                    ost[:, i * P : (i + 1) * P], cps[:], Id
                                )
                        if layer == 2:
                            nc.sync.dma_start(
                                out_t[q0 * P : (q0 + qc) * P, :]
                                .rearrange("(w l) g -> l w g", l=P),
                                ost[:, : qc * P]
                                .rearrange("l (w g) -> l w g", g=D),
                            )

                # ---- phase A: gather + transform + window aggregation ----
                cur_wins = {}
                stage = None
                pend = None  # (mk, chunk ids) 1-deep pipe
                rs_fired = []    # (group j, c0 when its sub-RS was emitted)
                cur_c0 = [0]
                combined = set()

                def drain(pend_batch):
                    nonlocal stage
                    mk, cids = pend_batch
                    sels = []
                    for c in cids:
                        for oi, wv, st, sp in ops_by_chunk.get(c, []):
                            sel = op_.tile([P, P], BF16, tag="sel")
                            nc.vector.tensor_scalar(
                                out=sel[:],
                                in0=iota_s[:],
                                scalar1=doff_s[:, oi : oi + 1],
                                scalar2=None,
                                op0=mybir.AluOpType.is_equal,
                            )
                            sels.append(sel)
                    si = 0
                    for i, c in enumerate(cids):
                        for oi, wv, st, sp in ops_by_chunk.get(c, []):
                            if st:
                                cur_wins[wv] = wpp.tile([P, P], F32, tag="win",
                                                        name="win",
                                                        space="PSUM")
                            nc.tensor.matmul(
                                out=cur_wins[wv][:],
                                lhsT=sels[si][:],
                                rhs=mk[:, i * P : (i + 1) * P],
                                start=st,
                                stop=sp,
                                skip_group_check=True,
                            )
                            si += 1
                            if sp:
                                w = wv
                                win = cur_wins.pop(wv)
                                if w % WB == 0:
                                    stage = sp_.tile([P, WB * P], BF16,
                                                     tag="st")
                                nc.scalar.activation(
                                    stage[:, (w % WB) * P : (w % WB + 1) * P],
                                    win[:],
                                    Id,
                                    scale=rvq_s[:, w : w + 1],
                                )
                                if w % WB == WB - 1 or w == NW - 1:
                                    w0 = (w // WB) * WB
                                    nc.sync.dma_start(
                                        PART[w0 * P : (w + 1) * P, :]
                                        .rearrange("(w l) g -> l w g", l=P),
                                        stage[:, : (w - w0 + 1) * P],
                                    )
                                    # fire sub-ReduceScatters whose PART
                                    # block just completed: overlaps the
                                    # collective with later groups' work
                                    if "RS" not in skip:
                                        for j, ge in enumerate(grp_end):
                                            if not (w0 <= ge <= w):
                                                continue
                                            pr0 = (ge + 1
                                                   - NCORES * gs[j]) * P
                                            or0 = sum(gs[:j]) * P
                                            nc.gpsimd.collective_compute(
                                                "ReduceScatter",
                                                mybir.AluOpType.add,
                                                replica_groups=[
                                                    list(range(NCORES))],
                                                ins=[PART[
                                                    pr0 : (ge + 1) * P,
                                                    :].opt()],
                                                outs=[RSOUT[
                                                    or0 : or0 + gs[j] * P,
                                                    :].opt()],
                                            )
                                            rs_fired.append((j, cur_c0[0]))

                for c0 in range(0, nch, gg) if "A" not in skip else []:
                    cur_c0[0] = c0
                    gcnt = min(gg, nch - c0)
                    gb = gp.tile([P, gcnt * P], BF16, tag="gb")
                    if "G" in skip:
                        nc.vector.memset(gb[:, 0:P], 0.0)
                    else:
                        nc.gpsimd.dma_gather(
                        out_ap=gb[:].rearrange("p (o e) -> p o e", o=1),
                        in_ap=tbl[:],
                        idxs_ap=idx_s[:, c0 * 8 : (c0 + gcnt) * 8],
                        num_idxs=gcnt * P,
                        num_idxs_reg=gcnt * P,
                        elem_size=D,
                        transpose=True,
                        sbuf_tokens_per_rank=P,
                        sbuf_free_dim_per_rank=2 * D,
                        sbuf_free_dim_pad_per_rank=0,
                        sbuf_byte_offset=0,
                        single_packet=gsp,
                        queue_num=(c0 // gg) % nq,
                        )
                    for s0 in range(0, gcnt, SUB):
                        scnt = min(SUB, gcnt - s0)
                        tfps = tfp.tile([P, SUB * P], F32, tag="tf", space="PSUM")
                        for i in range(scnt):
                            nc.tensor.matmul(
                                out=tfps[:, i * P : (i + 1) * P],
                                lhsT=gb[:, (s0 + i) * P : (s0 + i + 1) * P],
                                rhs=wl[:],
                                start=True,
                                stop=True,
                                skip_group_check=True,
                            )
                        mk = mp.tile([P, SUB * P], BF16, tag="mk")
                        nc.scalar.activation(
                            mk[:, : scnt * P], tfps[:, : scnt * P], Id
                        )
                        if pend is not None:
                            drain(pend)
                        pend = (mk, list(range(c0 + s0, c0 + s0 + scnt)))
                    # interleave combine of any RS group whose collective
                    # fired >= 2 gather batches ago (slack for the RS to
                    # land) so it overlaps the remaining aggregation
                    for j, fc0 in list(rs_fired):
                        if j not in combined and fc0 + 2 * gg <= c0:
                            combined.add(j)
                            rs_fired.remove((j, fc0))
                            emit_combine(j)
                if pend is not None:
                    drain(pend)
                    pend = None

                # ---- phase C tail: combine any groups not yet emitted ----
                for j in range(len(gs)):
                    if j not in combined:
                        combined.add(j)
                        emit_combine(j)
    nc.finalize()
    return nc


def kernel(x, edge_index, W_l1, b_l1, W_r1, W_l2, b_l2, W_r2):
    x = np.asarray(x, dtype=np.float32)
    cfg = Cfg(x.shape[0], np.asarray(edge_index).shape[1])
    prep = _host_prep(cfg, x, edge_index)

    iota = np.tile(np.arange(P, dtype=np.float32), (P, 1)).astype(BF)
    bt1 = np.tile(np.asarray(b_l1, np.float32) / P, (P, 1)).astype(BF)
    bt2 = np.tile(np.asarray(b_l2, np.float32) / P, (P, 1)).astype(BF)
    shared = dict(
        iota=iota,
        W_l1=np.asarray(W_l1, np.float32).astype(BF),
        W_r1=np.asarray(W_r1, np.float32).astype(BF),
        W_l2=np.asarray(W_l2, np.float32).astype(BF),
        W_r2=np.asarray(W_r2, np.float32).astype(BF),
        bt1=bt1,
        bt2=bt2,
    )
    in_maps = []
    for c in range(NCORES):
        in_maps.append(
            dict(
                shared,
                xmy=prep["xsl"][c],
                idxw=prep["idxw"][c],
                doffc=prep["doffc"][c],
                rvq=prep["rvq"],
            )
        )

    nc = _build_program(cfg, prep)
    res = run_bass_kernel_spmd(nc, in_maps, list(range(NCORES)))
    global LAST_EXEC_NS, LAST_RESULTS, LAST_NC, LAST_IN_MAPS, LAST_CFG, LAST_PREP
    LAST_EXEC_NS = res.exec_time_ns
    LAST_RESULTS = res
    LAST_NC = nc
    LAST_IN_MAPS = in_maps
    LAST_CFG = cfg
    LAST_PREP = prep

    out = np.empty((cfg.N, D), dtype=np.float32)
    for c in range(NCORES):
        out[c * cfg.NSH : (c + 1) * cfg.NSH] = (
            res.results[c]["out"][: cfg.NSH].astype(np.float32)
        )
    return out



# revision 49
# speedup vs baseline: 1.1833x; 1.1833x over previous
"""Trainium2 Bass kernel for the 2-layer GraphSAGE encoder (mean aggregation).

Computation (see reference):
  h   = relu(mean_agg(relu(x)[src] by dst) @ W_l1 + b_l1 + x @ W_r1)
  out =      mean_agg(h[src]       by dst) @ W_l2 + b_l2 + h @ W_r2

Distribution (src-sharded): nodes are sliced 1/8 per core; each core keeps
its slice's features resident in SBUF and handles exactly the edges whose
src falls in its slice.  Messages are fetched with SBUF-source dma_gather
(transpose mode -> feature-major chunks), transformed by W_l on the tensor
engine (linearity lets W_l and the 1/deg mean scale commute with the
segment sum), aggregated per 128-node dst window via one-hot matmuls into
PSUM, and written as bf16 partial sums for all N nodes.  Partial sums are
reduced across cores by PIPELINED sub-ReduceScatters: windows are processed
in an order that completes one cross-core block at a time, and each block's
collective fires as soon as its rows land in DRAM, overlapping the
collective with the remaining aggregation.  The combine (mean@W_l root
z@W_r + bias + activation) then runs purely per-slice, and layer 2 reuses
the resulting h slice as its gather table - no AllGather needed anywhere.

Host prep computes global in-degrees (the mean scale is folded into the
per-window flush as an activation scale) and per-core FLAT-PACKED edge
streams (dense 128-slot chunks, cores realigned only every WGRP windows,
~3% pad vs ~33% for per-window padding): int16 gather indices (wrap-16
layout) plus per-(chunk,window) uint8 dst-lane columns that drive is_equal
one-hot builds on the DVE, so per-core window straddling stays data-driven
while all cores share one instruction stream.
"""
import os
import sys

sys.path.insert(0, "/opt/trn_rl_repo")

import numpy as np
import ml_dtypes

import concourse.bacc as bacc
import concourse.tile as tile
from concourse import bass, mybir
from concourse.bass_utils import run_bass_kernel_spmd
from concourse.masks import make_identity

F32 = mybir.dt.float32
BF16 = mybir.dt.bfloat16
F8 = mybir.dt.float8e4
U8 = mybir.dt.uint8
I16 = mybir.dt.int16
BF = ml_dtypes.bfloat16
F8NP = ml_dtypes.float8_e4m3

P = 128
D = 128
NCORES = 8
PAD_LANE = 255.0   # doff value for pad slots (no iota lane matches)
G = 16             # chunks per dma_gather instruction
SUB = 4            # chunks per transform-psum batch
WB = 7             # windows per staging DMA batch

LAST_EXEC_NS = None
LAST_RESULTS = None
LAST_NC = None
LAST_IN_MAPS = None
LAST_CFG = None
LAST_PREP = None


class Cfg:
    def __init__(self, n_nodes, n_edges):
        assert n_nodes % NCORES == 0
        self.N = n_nodes
        self.E = n_edges
        self.NSH = n_nodes // NCORES            # real nodes per slice
        self.NWS = -(-self.NSH // P)            # windows per slice
        self.NSHP = self.NWS * P                # padded nodes per slice
        self.NW = NCORES * self.NWS             # global windows
        self.NPAD = NCORES * self.NSHP          # padded global nodes
        assert self.NSHP - 1 <= 32767


WGRP = 16          # windows per realignment group (flat-packed chunks inside)


def _host_prep(cfg, x, edge_index):
    """Flat-packed edge streams with periodic realignment.

    Edges are window-major sorted per core, then packed densely into
    128-slot chunks.  Cores realign only at WGRP-window group boundaries
    (pad-to-common-chunk-count there, ~3% waste), so a chunk may straddle
    window boundaries at per-core-varying positions.  The aggregation
    schedule is a flat list of ops (chunk, window, start, stop): each op
    builds a one-hot sel from its own doff column (255 = no-match for
    slots outside that window on that core) and matmuls into the window's
    PSUM accumulator, so per-core routing stays data-driven while the
    instruction stream is shared.
    """
    src = np.asarray(edge_index[0], dtype=np.int64)
    dst = np.asarray(edge_index[1], dtype=np.int64)
    E = src.shape[0]

    core = src // cfg.NSH
    idx16 = (src % cfg.NSH).astype(np.int16)
    pd = (dst // cfg.NSH) * cfg.NSHP + (dst % cfg.NSH)
    worig = pd // P
    lane = (pd % P).astype(np.int64)

    NW, NWS = cfg.NW, cfg.NWS
    # pi-order: windows processed grouped for pipelined sub-ReduceScatter.
    # RS group j covers local windows gs_lo[j]..gs_hi[j] of EVERY dst core;
    # processing order (j, dst_core, l) makes each group's PART rows complete
    # (and contiguous) early, so its collective can fire while later groups
    # still aggregate.  RSOUT comes out in plain l-ascending order per core.
    nsub = max(1, min(int(os.environ.get("GNN_NSUB", "7")), NWS))
    q, rem = divmod(NWS, nsub)
    gsizes = [q + (1 if j < rem else 0) for j in range(nsub)]
    pi_list = []                  # pi-rank -> global window id
    for j in range(nsub):
        lo = sum(gsizes[:j])
        for d in range(NCORES):
            for l in range(lo, lo + gsizes[j]):
                pi_list.append(d * NWS + l)
    rank_of_w = np.empty(NW, np.int64)
    rank_of_w[np.asarray(pi_list)] = np.arange(NW)
    grp_end = np.cumsum([NCORES * g for g in gsizes]) - 1   # pi-rank of group end

    w = rank_of_w[worig]          # everything downstream uses pi-ranks
    ngrp = -(-NW // WGRP)
    grp = w // WGRP

    # per-(core, group) counts -> chunks per group = max over cores
    cg = core * ngrp + grp
    counts = np.bincount(cg, minlength=NCORES * ngrp).reshape(NCORES, ngrp)
    Kg = np.maximum(-(-counts.max(axis=0) // P), 1)     # [ngrp] chunks
    gbase = np.zeros(ngrp + 1, np.int64)
    np.cumsum(Kg, out=gbase[1:])
    nch = int(gbase[-1])
    slots = nch * P

    # slot of each edge: window-major order within its (core, group)
    key = (core * ngrp + grp) * NW + w
    order = np.argsort(key, kind="stable")
    ks_cg = cg[order]
    gs = np.searchsorted(ks_cg, np.arange(NCORES * ngrp), side="left")
    rank = np.arange(E) - gs[ks_cg]
    slot = gbase[ks_cg % ngrp] * P + rank
    ecore = ks_cg // ngrp

    idx_st = np.zeros((NCORES, slots), np.int16)
    win_st = np.full((NCORES, slots), -1, np.int64)
    lan_st = np.full((NCORES, slots), int(PAD_LANE), np.uint8)
    idx_st[ecore, slot] = idx16[order]
    win_st[ecore, slot] = w[order]
    lan_st[ecore, slot] = lane[order].astype(np.uint8)

    # ops: per chunk, the union (over cores) of windows present
    wc = win_st.reshape(NCORES, nch, P)
    lc = lan_st.reshape(NCORES, nch, P)
    chunk_ws = []                 # per chunk: sorted windows present in any core
    for k in range(nch):
        ws = np.unique(wc[:, k, :])
        chunk_ws.append([int(v) for v in ws if v >= 0])
    # every window must flush exactly once; attach empty windows (possible
    # only in tiny graphs) to the first chunk of the next present window
    present = {wv for ws in chunk_ws for wv in ws}
    for wv in range(NW):
        if wv not in present:
            k = next((k for k in range(nch)
                      if any(x > wv for x in chunk_ws[k])), nch - 1)
            chunk_ws[k].append(wv)
            chunk_ws[k].sort()
    ops = []                      # [chunk, window, start, stop]
    first_op = {}
    last_op = {}
    for k in range(nch):
        for wv in chunk_ws[k]:
            op = len(ops)
            ops.append([k, wv, False, False])
            if wv not in first_op:
                first_op[wv] = op
            last_op[wv] = op
    for wv, op in first_op.items():
        ops[op][2] = True
    for wv, op in last_op.items():
        ops[op][3] = True
    nops = len(ops)

    doffc = np.full((NCORES, P, nops), int(PAD_LANE), np.uint8)
    for o, (k, wv, _, _) in enumerate(ops):
        m = wc[:, k, :] == wv                            # [8, 128]
        col = np.where(m, lc[:, k, :], int(PAD_LANE))
        doffc[:, :, o] = col

    idxw = np.ascontiguousarray(
        idx_st.reshape(NCORES, slots // 16, 16).transpose(0, 2, 1)
    )                                                   # [8, 16, slots/16]

    indeg = np.bincount(dst, minlength=cfg.N).astype(np.float64)
    rv = (1.0 / np.maximum(indeg, 1.0)).astype(np.float32)
    rvp = np.ones(cfg.NPAD, np.float32)
    nodes = np.arange(cfg.N)
    rvp[(nodes // cfg.NSH) * cfg.NSHP + nodes % cfg.NSH] = rv
    rvq = np.ascontiguousarray(
        rvp.reshape(cfg.NW, P)[np.asarray(pi_list)].T
    ).astype(BF)                                        # [128, NW] pi-order

    xsl = np.zeros((NCORES, cfg.NSHP, D), BF)
    xs = np.asarray(x, np.float32).reshape(NCORES, cfg.NSH, D)
    xsl[:, : cfg.NSH] = xs

    return dict(ops=ops, nch=nch, idxw=idxw, doffc=doffc, rvq=rvq, xsl=xsl,
                grp_end=[int(v) for v in grp_end], gs=gsizes)


def _build_program(cfg, prep):
    ops, nch = prep["ops"], prep["nch"]
    grp_end, gs = prep["grp_end"], prep["gs"]
    skip = os.environ.get("GNN_SKIP", "")
    gg = int(os.environ.get("GNN_G", G))
    gsp = bool(int(os.environ.get("GNN_SP", "0")))
    nrep = int(os.environ.get("GNN_REPEAT", "1"))
    nq = int(os.environ.get("GNN_NQ", "1"))
    nc = bacc.Bacc(None, target_bir_lowering=False, debug=False,
                   num_swdge_queues=nq)
    slots = nch * P
    nops = len(ops)
    NWS, NW = cfg.NWS, cfg.NW
    ops_by_chunk = {}
    for oi, (k, wv, st, sp) in enumerate(ops):
        ops_by_chunk.setdefault(k, []).append((oi, wv, st, sp))

    xsl_t = nc.declare_dram_parameter("xmy", [cfg.NSHP, D], BF16, isOutput=False)
    idxw_t = nc.declare_dram_parameter("idxw", [16, slots // 16], I16, isOutput=False)
    doff_t = nc.declare_dram_parameter("doffc", [P, nops], U8, isOutput=False)
    rvq_t = nc.declare_dram_parameter("rvq", [P, NW], BF16, isOutput=False)
    iota_t = nc.declare_dram_parameter("iota", [P, P], BF16, isOutput=False)
    wl1_t = nc.declare_dram_parameter("W_l1", [D, D], BF16, isOutput=False)
    wr1_t = nc.declare_dram_parameter("W_r1", [D, D], BF16, isOutput=False)
    wl2_t = nc.declare_dram_parameter("W_l2", [D, D], BF16, isOutput=False)
    wr2_t = nc.declare_dram_parameter("W_r2", [D, D], BF16, isOutput=False)
    bt1_t = nc.declare_dram_parameter("bt1", [P, P], BF16, isOutput=False)
    bt2_t = nc.declare_dram_parameter("bt2", [P, P], BF16, isOutput=False)
    out_t = nc.declare_dram_parameter("out", [cfg.NSHP, D], BF16, isOutput=True)

    Id = mybir.ActivationFunctionType.Identity
    Relu = mybir.ActivationFunctionType.Relu

    with tile.TileContext(nc, trace_sim=bool(os.environ.get("GNN_TRACE_SIM"))) as tc:
        with (
            tc.tile_pool(name="const", bufs=1) as cp,
            tc.tile_pool(name="gather", bufs=3) as gp,
            tc.tile_pool(name="mk", bufs=3) as mp,
            tc.tile_pool(name="sel", bufs=8) as op_,
            tc.tile_pool(name="stage", bufs=2) as sp_,
            tc.tile_pool(name="rs", bufs=1) as rp,
            tc.tile_pool(name="tf", bufs=2, space="PSUM") as tfp,
            tc.tile_pool(name="win", bufs=4, space="PSUM") as wpp,
            tc.tile_pool(name="tps", bufs=2, space="PSUM") as tpp,
            tc.tile_pool(name="dram", bufs=1, space="DRAM") as dp,
        ):
            ident = cp.tile([P, P], BF16)
            make_identity(nc, ident[:])
            ones_t = cp.tile([P, P], BF16)
            nc.vector.memset(ones_t[:], 1.0)
            iota_s = cp.tile([P, P], BF16)
            nc.sync.dma_start(iota_s[:], iota_t[:, :])
            wl1 = cp.tile([D, D], BF16)
            nc.sync.dma_start(wl1[:], wl1_t[:, :])
            wr1 = cp.tile([D, D], BF16)
            nc.sync.dma_start(wr1[:], wr1_t[:, :])
            wl2 = cp.tile([D, D], BF16)
            nc.sync.dma_start(wl2[:], wl2_t[:, :])
            wr2 = cp.tile([D, D], BF16)
            nc.sync.dma_start(wr2[:], wr2_t[:, :])
            bt1 = cp.tile([P, P], BF16)
            nc.sync.dma_start(bt1[:], bt1_t[:, :])
            bt2 = cp.tile([P, P], BF16)
            nc.sync.dma_start(bt2[:], bt2_t[:, :])
            rvq_b = cp.tile([P, NW], BF16)
            nc.sync.dma_start(rvq_b[:], rvq_t[:, :])
            rvq_s = cp.tile([P, NW], F32)
            nc.vector.tensor_copy(rvq_s[:], rvq_b[:])
            doff_b = cp.tile([P, nops], U8)
            nc.sync.dma_start(doff_b[:], doff_t[:, :])
            doff_s = cp.tile([P, nops], F32)
            nc.vector.tensor_copy(doff_s[:], doff_b[:])
            idx_s = cp.tile([P, slots // 16], I16)
            for r in range(8):
                nc.sync.dma_start(idx_s[16 * r : 16 * (r + 1), :], idxw_t[:, :])
            tbl1 = cp.tile([P, NWS * D], BF16)
            nc.sync.dma_start(
                tbl1[:].rearrange("t (r e) -> t r e", e=D),
                xsl_t[:, :].rearrange("(r t) e -> t r e", t=P),
            )
            tc.strict_bb_all_engine_barrier()

            # feature-major views of the slice (roots), gather table for L2
            xT = cp.tile([P, NWS * P], BF16)
            hT = cp.tile([P, NWS * P], BF16)
            tbl2 = cp.tile([P, NWS * D], BF16)
            for w in range(NWS):
                tp = tpp.tile([P, P], BF16, tag="tps", space="PSUM")
                nc.tensor.transpose(
                    out=tp[:], in_=tbl1[:, w * P : (w + 1) * P], identity=ident[:]
                )
                nc.scalar.activation(xT[:, w * P : (w + 1) * P], tp[:], Id)
            # messages are relu(x): relu the gather table in place (roots
            # already captured in xT)
            nc.scalar.activation(tbl1[:], tbl1[:], Relu)

            PART = dp.tile([cfg.NPAD, D], BF16, name="part")
            RSOUT = dp.tile([cfg.NSHP, D], BF16, name="rsout")

            for layer in [l for _ in range(nrep) for l in (1, 2)]:
                tbl = tbl1 if layer == 1 else tbl2
                zT = xT if layer == 1 else hT
                wl = wl1 if layer == 1 else wl2
                wr = wr1 if layer == 1 else wr2
                bt = bt1 if layer == 1 else bt2

                # ---- phase A: gather + transform + window aggregation ----
                cur_wins = {}
                stage = None
                pend = None  # (mk, chunk ids) 1-deep pipe

                def drain(pend_batch):
                    nonlocal stage
                    mk, cids = pend_batch
                    sels = []
                    for c in cids:
                        for oi, wv, st, sp in ops_by_chunk.get(c, []):
                            sel = op_.tile([P, P], BF16, tag="sel")
                            nc.vector.tensor_scalar(
                                out=sel[:],
                                in0=iota_s[:],
                                scalar1=doff_s[:, oi : oi + 1],
                                scalar2=None,
                                op0=mybir.AluOpType.is_equal,
                            )
                            sels.append(sel)
                    si = 0
                    for i, c in enumerate(cids):
                        for oi, wv, st, sp in ops_by_chunk.get(c, []):
                            if st:
                                cur_wins[wv] = wpp.tile([P, P], F32, tag="win",
                                                        name="win",
                                                        space="PSUM")
                            nc.tensor.matmul(
                                out=cur_wins[wv][:],
                                lhsT=sels[si][:],
                                rhs=mk[:, i * P : (i + 1) * P],
                                start=st,
                                stop=sp,
                                skip_group_check=True,
                            )
                            si += 1
                            if sp:
                                w = wv
                                win = cur_wins.pop(wv)
                                if w % WB == 0:
                                    stage = sp_.tile([P, WB * P], BF16,
                                                     tag="st")
                                nc.scalar.activation(
                                    stage[:, (w % WB) * P : (w % WB + 1) * P],
                                    win[:],
                                    Id,
                                    scale=rvq_s[:, w : w + 1],
                                )
                                if w % WB == WB - 1 or w == NW - 1:
                                    w0 = (w // WB) * WB
                                    nc.sync.dma_start(
                                        PART[w0 * P : (w + 1) * P, :]
                                        .rearrange("(w l) g -> l w g", l=P),
                                        stage[:, : (w - w0 + 1) * P],
                                    )
                                    # fire sub-ReduceScatters whose PART
                                    # block just completed: overlaps the
                                    # collective with later groups' work
                                    if "RS" not in skip:
                                        for j, ge in enumerate(grp_end):
                                            if not (w0 <= ge <= w):
                                                continue
                                            pr0 = (ge + 1
                                                   - NCORES * gs[j]) * P
                                            or0 = sum(gs[:j]) * P
                                            nc.gpsimd.collective_compute(
                                                "ReduceScatter",
                                                mybir.AluOpType.add,
                                                replica_groups=[
                                                    list(range(NCORES))],
                                                ins=[PART[
                                                    pr0 : (ge + 1) * P,
                                                    :].opt()],
                                                outs=[RSOUT[
                                                    or0 : or0 + gs[j] * P,
                                                    :].opt()],
                                            )

                for c0 in range(0, nch, gg) if "A" not in skip else []:
                    gcnt = min(gg, nch - c0)
                    gb = gp.tile([P, gcnt * P], BF16, tag="gb")
                    if "G" in skip:
                        nc.vector.memset(gb[:, 0:P], 0.0)
                    else:
                        nc.gpsimd.dma_gather(
                        out_ap=gb[:].rearrange("p (o e) -> p o e", o=1),
                        in_ap=tbl[:],
                        idxs_ap=idx_s[:, c0 * 8 : (c0 + gcnt) * 8],
                        num_idxs=gcnt * P,
                        num_idxs_reg=gcnt * P,
                        elem_size=D,
                        transpose=True,
                        sbuf_tokens_per_rank=P,
                        sbuf_free_dim_per_rank=2 * D,
                        sbuf_free_dim_pad_per_rank=0,
                        sbuf_byte_offset=0,
                        single_packet=gsp,
                        queue_num=(c0 // gg) % nq,
                        )
                    for s0 in range(0, gcnt, SUB):
                        scnt = min(SUB, gcnt - s0)
                        tfps = tfp.tile([P, SUB * P], F32, tag="tf", space="PSUM")
                        for i in range(scnt):
                            nc.tensor.matmul(
                                out=tfps[:, i * P : (i + 1) * P],
                                lhsT=gb[:, (s0 + i) * P : (s0 + i + 1) * P],
                                rhs=wl[:],
                                start=True,
                                stop=True,
                                skip_group_check=True,
                            )
                        mk = mp.tile([P, SUB * P], BF16, tag="mk")
                        nc.scalar.activation(
                            mk[:, : scnt * P], tfps[:, : scnt * P], Id
                        )
                        if pend is not None:
                            drain(pend)
                        pend = (mk, list(range(c0 + s0, c0 + s0 + scnt)))
                if pend is not None:
                    drain(pend)
                    pend = None

                # (sub-ReduceScatters were fired from drain() as their
                # PART blocks completed)

                # ---- phase C: combine own slice ----
                for q0 in range(0, NWS, WB) if "C" not in skip else []:
                    qc = min(WB, NWS - q0)
                    rs = rp.tile([P, WB * P], BF16, tag="rs")
                    nc.sync.dma_start(
                        rs[:, : qc * P].rearrange("l (w g) -> l w g", g=D),
                        RSOUT[q0 * P : (q0 + qc) * P, :]
                        .rearrange("(w l) g -> l w g", l=P),
                    )
                    if layer == 2:
                        ost = sp_.tile([P, WB * P], BF16, tag="ost", name="ost")
                    else:
                        ost = None
                    for i in range(qc):
                        w = q0 + i
                        cps = wpp.tile([P, P], F32, tag="win", name="cps",
                                       space="PSUM")
                        nc.tensor.matmul(
                            out=cps[:], lhsT=ident[:],
                            rhs=rs[:, i * P : (i + 1) * P],
                            start=True, stop=False, skip_group_check=True,
                        )
                        nc.tensor.matmul(
                            out=cps[:], lhsT=ones_t[:], rhs=bt[:],
                            start=False, stop=False, skip_group_check=True,
                        )
                        nc.tensor.matmul(
                            out=cps[:], lhsT=zT[:, w * P : (w + 1) * P],
                            rhs=wr[:],
                            start=False, stop=True, skip_group_check=True,
                        )
                        if layer == 1:
                            nc.scalar.activation(
                                tbl2[:, w * P : (w + 1) * P], cps[:], Relu
                            )
                            tp = tpp.tile([P, P], BF16, tag="tps", space="PSUM")
                            nc.tensor.transpose(
                                out=tp[:], in_=tbl2[:, w * P : (w + 1) * P],
                                identity=ident[:],
                            )
                            nc.scalar.activation(
                                hT[:, w * P : (w + 1) * P], tp[:], Id
                            )
                        else:
                            nc.scalar.activation(
                                ost[:, i * P : (i + 1) * P], cps[:], Id
                            )
                    if layer == 2:
                        nc.sync.dma_start(
                            out_t[q0 * P : (q0 + qc) * P, :]
                            .rearrange("(w l) g -> l w g", l=P),
                            ost[:, : qc * P].rearrange("l (w g) -> l w g", g=D),
                        )
    nc.finalize()
    return nc


def kernel(x, edge_index, W_l1, b_l1, W_r1, W_l2, b_l2, W_r2):
    x = np.asarray(x, dtype=np.float32)
    cfg = Cfg(x.shape[0], np.asarray(edge_index).shape[1])
    prep = _host_prep(cfg, x, edge_index)

    iota = np.tile(np.arange(P, dtype=np.float32), (P, 1)).astype(BF)
    bt1 = np.tile(np.asarray(b_l1, np.float32) / P, (P, 1)).astype(BF)
    bt2 = np.tile(np.asarray(b_l2, np.float32) / P, (P, 1)).astype(BF)
    shared = dict(
        iota=iota,
        W_l1=np.asarray(W_l1, np.float32).astype(BF),
        W_r1=np.asarray(W_r1, np.float32).astype(BF),
        W_l2=np.asarray(W_l2, np.float32).astype(BF),
        W_r2=np.asarray(W_r2, np.float32).astype(BF),
        bt1=bt1,
        bt2=bt2,
    )
    in_maps = []
    for c in range(NCORES):
        in_maps.append(
            dict(
                shared,
                xmy=prep["xsl"][c],
                idxw=prep["idxw"][c],
                doffc=prep["doffc"][c],
                rvq=prep["rvq"],
            )
        )

    nc = _build_program(cfg, prep)
    res = run_bass_kernel_spmd(nc, in_maps, list(range(NCORES)))
    global LAST_EXEC_NS, LAST_RESULTS, LAST_NC, LAST_IN_MAPS, LAST_CFG, LAST_PREP
    LAST_EXEC_NS = res.exec_time_ns
    LAST_RESULTS = res
    LAST_NC = nc
    LAST_IN_MAPS = in_maps
    LAST_CFG = cfg
    LAST_PREP = prep

    out = np.empty((cfg.N, D), dtype=np.float32)
    for c in range(NCORES):
        out[c * cfg.NSH : (c + 1) * cfg.NSH] = (
            res.results[c]["out"][: cfg.NSH].astype(np.float32)
        )
    return out

